# revision 35
# baseline (speedup 1.0000x reference)
"""Multi-head causal attention (B=2, T=2048, C=1024, H=16, HS=64) on 8 TRN2
NeuronCores.

Sharding: hybrid batch x head-group tensor parallel. Core c handles batch
c//4 and heads 4*(c%4) .. 4*(c%4)+3, processed as two head-PAIRS (A, B)
that are software-pipelined against each other. Each core loads only its
batch's activations once (bf16, SBUF-resident), computes a partial output
out_t[C, T] via its 256 rows of w_proj, and the host sums 4 partials per
batch (+ bias) and transposes.

Per-core kernel (bf16 matmuls, fp32 PSUM accumulate):
  - All inputs host-pre-arranged partition-major so each DMA descriptor
    moves a 4-32 KB contiguous row (the DMA was descriptor-bound, not
    bandwidth-bound, in the naive layout).
  - Warm-up matmuls on the first-arriving weight sliver keep the PE HAM
    clock at 2.4 GHz through the initial DMA fill (and through known
    stall windows); a dummy exp preloads the ScalarE activation table.
  - QT/KT/VT [128(2 heads x 64), T] per pair: lhsT=weight chunks (bf16,
    FWL-fast weight load), rhs=resident xT chunks; 8-matmul runs hit the
    216 ns/matmul N=512 streaming floor.
  - V_aug [keys, 128]: V (cols 0:64 via PE-transpose of VT) | ones.
  - Flash-style causal attention in transposed layout: S^T[keys, q] via
    lhsT=KT block (K=64; the two heads' matmuls occupy disjoint PE row
    groups and overlap), exp on ScalarE, O^T accumulated via lhsT=V_aug
    — rows 64:128 give the softmax sums. OT for block-pair jg is emitted
    during jg+1 so the PE never head-of-line blocks on exp.
  - Generator-based schedule: each attention group's exp-bound steps are
    filled with QKV units of the other pair or (transposed-output)
    projection units, with explicit prerequisite drains.
  - PSUM is only read via copies (ALU ops reading PSUM mis-execute) and
    partition shifts only happen in copies (lane-wise ALU ops cannot
    shift partitions).
"""

import math
import sys
from contextlib import ExitStack

if "/opt/trn_rl_repo" not in sys.path:
    sys.path.insert(0, "/opt/trn_rl_repo")

import numpy as np

import concourse.mybir as mybir
import concourse.tile as tile
from concourse import bacc
from concourse.bass import ts
from concourse.bass_utils import run_bass_kernel_spmd

B, T, C = 2, 2048, 1024
H, HS = 16, 64
NCORES = 8
P = 128
G = 512  # q-group size
NG = T // G
KB = 128  # key block
NPO = C // P  # contraction chunks
F32 = mybir.dt.float32
BF16 = mybir.dt.bfloat16
EXP = mybir.ActivationFunctionType.Exp
MULT = mybir.AluOpType.mult

_nc_cache = {}


def _emit(tc):
    nc = tc.nc
    # All inputs are host-pre-arranged so every DMA descriptor moves a
    # 4-32 KB contiguous row (descriptor-count, not bandwidth, limits the
    # small strided layout).
    xt = nc.dram_tensor("xt", [P, NG, NPO, G], BF16, kind="ExternalInput").ap()
    wq2 = nc.dram_tensor("wq2", [P, NPO, 2 * P], BF16, kind="ExternalInput").ap()
    wk2 = nc.dram_tensor("wk2", [P, NPO, 2 * P], BF16, kind="ExternalInput").ap()
    wv2 = nc.dram_tensor("wv2", [P, NPO, 2 * P], BF16, kind="ExternalInput").ap()
    wptd = nc.dram_tensor("wpt", [P, 2, C], BF16, kind="ExternalInput").ap()
    trid = nc.dram_tensor("tri", [P, P], BF16, kind="ExternalInput").ap()
    identd = nc.dram_tensor("ident", [P, 64], BF16, kind="ExternalInput").ap()
    onesd = nc.dram_tensor("ones", [P, T // KB, 64], BF16, kind="ExternalInput").ap()
    out = nc.dram_tensor("out", [T, C], F32, kind="ExternalOutput").ap()

    ctx = ExitStack()
    persist = ctx.enter_context(tc.tile_pool(name="persist", bufs=1))
    qk_pool = ctx.enter_context(tc.tile_pool(name="qkp", bufs=2))
    vt_pool = ctx.enter_context(tc.tile_pool(name="vtp", bufs=2))
    vaug_pool = ctx.enter_context(tc.tile_pool(name="vaugp", bufs=2))
    pt_pool = ctx.enter_context(tc.tile_pool(name="ptp", bufs=3))
    norm_pool = ctx.enter_context(tc.tile_pool(name="normp", bufs=2))
    ohat_pool = ctx.enter_context(tc.tile_pool(name="ohatp", bufs=2))
    out_pool = ctx.enter_context(tc.tile_pool(name="outp", bufs=4))
    st_psum = ctx.enter_context(tc.tile_pool(name="stps", bufs=2, space="PSUM"))
    ot_psum = ctx.enter_context(tc.tile_pool(name="otps", bufs=2, space="PSUM"))
    mm_psum = ctx.enter_context(tc.tile_pool(name="mmps", bufs=2, space="PSUM"))

    xt_sb = persist.tile([P, NG, NPO, G], BF16, tag="xt")
    wq_sb = persist.tile([P, NPO, 2 * P], BF16, tag="wq")
    wk_sb = persist.tile([P, NPO, 2 * P], BF16, tag="wk")
    wv_sb = persist.tile([P, NPO, 2 * P], BF16, tag="wv")
    wpt_sb = persist.tile([P, 2, C], BF16, tag="wpt")
    tri_sb = persist.tile([P, P], BF16, tag="tri")
    ident = persist.tile([P, 64], BF16, tag="ident")

    # ---- DMA schedule: a sliver of wq first (enables warm-up), then xt ----
    nc.sync.dma_start(wq_sb[:, 0:2, :], wq2[:, 0:2, :])

    # PE warm-up during the DMA fill: keeps the HAM clock releasing to
    # 2.4 GHz before the first real matmul. Also preload the exp table.
    warm_ps = mm_psum.tile([P, G], F32, tag="mm", name="warm")
    for _ in range(15):
        nc.tensor.matmul(
            warm_ps[:], wq_sb[:, 0, 0:P], wq_sb[:, 0:2, :], start=True, stop=True
        )
    dummy_act = norm_pool.tile([P, 8], F32, tag="dact", name="dummy_act")
    nc.scalar.activation(dummy_act[:], wq_sb[:, 0, 0:8], EXP, scale=0.01)

    # xt in po-halves: subtile deps let each QKV matmul start as soon as
    # the half covering its contraction chunk has landed.
    nc.sync.dma_start(xt_sb[:, 0, 0:4, :], xt[:, 0, 0:4, :])
    nc.gpsimd.dma_start(xt_sb[:, 0, 4:NPO, :], xt[:, 0, 4:NPO, :])
    nc.sync.dma_start(wq_sb[:, 2:NPO, :], wq2[:, 2:NPO, :])
    nc.sync.dma_start(wk_sb[:], wk2[:])
    nc.sync.dma_start(wv_sb[:], wv2[:])
    for tg in range(1, NG):
        nc.sync.dma_start(xt_sb[:, tg, 0:4, :], xt[:, tg, 0:4, :])
        nc.gpsimd.dma_start(xt_sb[:, tg, 4:NPO, :], xt[:, tg, 4:NPO, :])
    nc.gpsimd.dma_start(ident[:], identd[:])
    nc.gpsimd.dma_start(tri_sb[:], trid[:])

    def new_pair(p):
        # ones DMAs (needed only by this pair's first OT) are deferred out
        # of the startup window via emit_ones.
        st = {
            "p": p,
            "qt": qk_pool.tile([P, T], BF16, tag="qt", name=f"qt{p}"),
            "kt": qk_pool.tile([P, T], BF16, tag="kt", name=f"kt{p}"),
            "vt": vt_pool.tile([P, T], BF16, tag="vt", name=f"vt{p}"),
            "ohat": ohat_pool.tile([P, T], BF16, tag="ohat", name=f"oh{p}"),
            "vaug": [
                vaug_pool.tile([P, T // KB, 128], BF16, tag=f"vaug{h}", name=f"va{p}{h}")
                for h in range(2)
            ],
        }
        return st

    def emit_ones(st):
        for h in range(2):
            nc.gpsimd.dma_start(st["vaug"][h][:, :, 64:128], onesd[:])

    stA = new_pair(0)
    stB = new_pair(1)
    emit_ones(stA)

    def emit_qkv(st, which, tg):
        w_sb, dst = {
            "q": (wq_sb, st["qt"]),
            "k": (wk_sb, st["kt"]),
            "v": (wv_sb, st["vt"]),
        }[which]
        sl = slice(P * st["p"], P * st["p"] + P)
        ps = mm_psum.tile([P, G], F32, tag="mm", name=f"qkv{which}{tg}")
        for po in range(NPO):
            nc.tensor.matmul(
                ps[:],
                w_sb[:, po, sl],
                xt_sb[:, tg, po, :],
                start=(po == 0),
                stop=(po == NPO - 1),
            )
        nc.vector.tensor_copy(dst[:, ts(tg, G)], ps[:])

    def emit_vaug(st, tg):
        tps = [
            mm_psum.tile([P, 4, 64], BF16, tag="mm", name=f"vtr{h}")
            for h in range(2)
        ]
        for kk in range(4):
            kb = 4 * tg + kk
            for h in range(2):
                nc.tensor.transpose(
                    tps[h][:, kk, :],
                    st["vt"][64 * h : 64 * h + 64, ts(kb, KB)],
                    ident[64 * h : 64 * h + 64, :],
                )
        for h in range(2):
            nc.vector.tensor_copy(st["vaug"][h][:, 4 * tg : 4 * tg + 4, 0:64], tps[h][:])

    def emit_ot(st, g, otps, js, pt_h):
        n_j = 4 * g + 4
        for idx, j in enumerate(js):
            r = j - 4 * g
            q0 = P * r if r >= 0 else 0
            for h in range(2):
                nc.tensor.matmul(
                    otps[h][:, q0:G],
                    st["vaug"][h][:, j, :],
                    pt_h[h][:, idx, q0:G],
                    start=(j == 0),
                    stop=(j == n_j - 1),
                )

    def gen_attn(st, g):
        p, qt, kt, ohat = st["p"], st["qt"], st["kt"], st["ohat"]
        n_j = 4 * g + 4
        njg = (n_j + 1) // 2
        otps = [
            ot_psum.tile([P, G], F32, tag="ot", name=f"ot{p}{g}{h}") for h in range(2)
        ]
        prev = None
        for jg in range(njg):
            js = [j for j in (2 * jg, 2 * jg + 1) if j < n_j]
            diag = 2 * jg >= 4 * g
            stps_h = [
                st_psum.tile([P, 2, G], F32, tag="st", name=f"st{p}{g}{h}")
                for h in range(2)
            ]
            pt_h = [
                pt_pool.tile([P, 2, G], BF16, tag=f"pt{h}", name=f"pt{p}{g}{h}")
                for h in range(2)
            ]
            for idx, j in enumerate(js):
                r = j - 4 * g
                q0 = P * r if r >= 0 else 0
                for h in range(2):
                    hb = 64 * h
                    nc.tensor.matmul(
                        stps_h[h][:, idx, q0:G],
                        kt[hb : hb + 64, ts(j, KB)],
                        qt[hb : hb + 64, G * g + q0 : G * (g + 1)],
                        start=True,
                        stop=True,
                    )
            for h in range(2):
                if not diag:
                    nc.scalar.activation(
                        pt_h[h][:, :, :],
                        stps_h[h][:, :, :],
                        EXP,
                        scale=float(HS) ** -0.5,
                    )
                else:
                    # one exp per (jg, head): the extra [q00:q01) slice of
                    # idx 1 is stale-but-bounded PSUM whose pt columns the
                    # OT matmul never reads.
                    q00 = P * (js[0] - 4 * g)
                    nc.scalar.activation(
                        pt_h[h][:, :, q00:G],
                        stps_h[h][:, :, q00:G],
                        EXP,
                        scale=float(HS) ** -0.5,
                    )
                    for idx, j in enumerate(js):
                        q0 = P * (j - 4 * g)
                        nc.vector.tensor_tensor(
                            pt_h[h][:, idx, q0 : q0 + P],
                            pt_h[h][:, idx, q0 : q0 + P],
                            tri_sb[:],
                            MULT,
                        )
            if prev is not None:
                emit_ot(st, g, otps, *prev)
            prev = (js, pt_h)
            yield
        emit_ot(st, g, otps, *prev)
        # normalization: PSUM is only ever read via copies (ALU ops reading
        # PSUM directly mis-execute); copies also handle partition shifts.
        l_sb = norm_pool.tile([P, G], F32, tag="lsb", name=f"l{p}{g}")
        rinv = norm_pool.tile([P, G], F32, tag="rinv", name=f"r{p}{g}")
        stag = norm_pool.tile([P, G], F32, tag="stag", name=f"sg{p}{g}")
        for h in range(2):
            hb = 64 * h
            nc.scalar.copy(l_sb[hb : hb + 64, :], otps[h][64:128, :])
            nc.vector.tensor_copy(stag[hb : hb + 64, :], otps[h][0:64, :])
        nc.vector.reciprocal_approx_fast(rinv[:], l_sb[:])
        nc.vector.tensor_tensor(ohat[:, ts(g, G)], stag[:], rinv[:], MULT)
        yield

    def gen_qkv_tg(st, tg):
        for which in ("q", "k", "v"):
            emit_qkv(st, which, tg)
            yield
        emit_vaug(st, tg)
        yield

    def gen_proj(tg):
        # lhsT = ohat t-chunk (stationary, reused across both c-halves),
        # rhs = w_projT moving: halves the proj LDWEIGHTS count and gives
        # the output in [T, C] orientation directly.
        for tc4 in range(G // P):
            t0 = G * tg + P * tc4
            pss = [
                mm_psum.tile([P, G], F32, tag="mm", name=f"pj{tg}{tc4}{n}")
                for n in range(2)
            ]
            for pp in range(2):
                for n in range(2):
                    nc.tensor.matmul(
                        pss[n][:],
                        [stA, stB][pp]["ohat"][:, t0 : t0 + P],
                        wpt_sb[:, pp, ts(n, G)],
                        start=(pp == 0),
                        stop=(pp == 1),
                    )
            for n in range(2):
                o_sb = out_pool.tile([P, G], F32, tag="osb", name=f"osb{tg}{tc4}{n}")
                # mid-kernel: keep ScalarE free for exp; at the tail split
                # copies across engines to drain faster.
                if tg == 3 and n == 1:
                    nc.scalar.copy(o_sb[:], pss[n][:])
                else:
                    nc.vector.tensor_copy(o_sb[:], pss[n][:])
                if n == 0:
                    nc.sync.dma_start(out[t0 : t0 + P, ts(n, G)], o_sb[:])
                else:
                    nc.gpsimd.dma_start(out[t0 : t0 + P, ts(n, G)], o_sb[:])
            yield

    def advance(gens):
        while gens:
            try:
                next(gens[0])
                return
            except StopIteration:
                gens.pop(0)

    def drain(gens):
        while gens:
            advance(gens)

    # ---- pipelined schedule: attention groups start as soon as their
    # inputs exist; later QKV and proj units serve as PE filler inside
    # each group's exp-bound steps. Every group's prerequisite QKV units
    # are force-drained before the group starts. ----
    fill = []

    def attn_with_fill(st, g, late=None):
        # `late` generators join the fill only after the first step, so a
        # unit stalling on the previous group's norm can't head-of-line
        # block this group's first score matmuls.
        for i, _ in enumerate(gen_attn(st, g)):
            advance(fill)
            if i == 0 and late:
                fill.extend(late)

    qA = [gen_qkv_tg(stA, tg) for tg in range(NG)]
    qB = [gen_qkv_tg(stB, tg) for tg in range(NG)]
    pj = [gen_proj(tg) for tg in range(NG)]

    def emit_warm(n):
        # Filler matmuls on already-loaded weights: executed in queue order,
        # they occupy the PE during xt-DMA stalls and keep the HAM clock hot.
        wps = mm_psum.tile([P, G], F32, tag="mm", name="warmf")
        for _ in range(n):
            nc.tensor.matmul(
                wps[:], wq_sb[:, 0, 0:P], wq_sb[:, 0:2, :], start=True, stop=True
            )

    drain([qA[0]])
    emit_warm(8)
    fill.append(qA[1])
    attn_with_fill(stA, 0)
    drain([qA[1]])
    emit_warm(6)
    emit_ones(stB)
    fill.append(qA[2])
    attn_with_fill(stA, 1)
    drain([qA[2]])
    fill.append(qA[3])
    attn_with_fill(stA, 2)
    drain([qA[3]])
    nc.gpsimd.dma_start(wpt_sb[:], wptd[:])
    fill.append(qB[0])
    fill.append(qB[1])
    attn_with_fill(stA, 3)
    drain([qB[0]])
    fill.append(qB[2])
    fill.append(qB[3])
    attn_with_fill(stB, 0)
    drain([qB[1]])
    attn_with_fill(stB, 1, late=[pj[0]])
    drain([qB[2]])
    attn_with_fill(stB, 2, late=[pj[1]])
    drain([qB[3]])
    attn_with_fill(stB, 3, late=[pj[2]])
    fill.append(pj[3])
    drain(fill)

    ctx.close()


def _build():
    if "nc" in _nc_cache:
        return _nc_cache["nc"]
    nc = bacc.Bacc("TRN2", target_bir_lowering=False, debug=False)
    with tile.TileContext(nc) as tc:
        _emit(tc)
    nc.compile()
    _nc_cache["nc"] = nc
    return nc


def _make_in_maps(x, wq, wk, wv, w_proj):
    import ml_dtypes

    bf = ml_dtypes.bfloat16
    tri = np.triu(np.ones((P, P), dtype=np.float32)).astype(bf)
    ident = np.tile(np.eye(64, dtype=np.float32), (2, 1)).astype(bf)
    ones = np.ones((P, T // KB, 64), dtype=np.float32).astype(bf)

    def part_major(a, blocks):
        # [blocks*P, cols] -> [P, blocks, cols] so each partition's DMA
        # row is one contiguous span.
        cols = a.shape[1]
        return np.ascontiguousarray(
            a.reshape(blocks, P, cols).transpose(1, 0, 2)
        ).astype(bf)

    xts = []
    for b in range(B):
        xT = np.asarray(x[b], np.float32).T  # [C, T]
        xts.append(
            np.ascontiguousarray(
                xT.reshape(NPO, P, NG, G).transpose(1, 2, 0, 3)
            ).astype(bf)
        )
    in_maps = []
    for c in range(NCORES):
        b, hp = divmod(c, 4)
        h0 = 4 * hp
        in_maps.append(
            {
                "xt": xts[b],
                "wq2": part_major(
                    np.concatenate([wq[h0 + i] for i in range(4)], axis=1), NPO
                ),
                "wk2": part_major(
                    np.concatenate([wk[h0 + i] for i in range(4)], axis=1), NPO
                ),
                "wv2": part_major(
                    np.concatenate([wv[h0 + i] for i in range(4)], axis=1), NPO
                ),
                "wpt": part_major(
                    np.ascontiguousarray(w_proj[:, 256 * hp : 256 * (hp + 1)].T), 2
                ),
                "tri": tri,
                "ident": ident,
                "ones": ones,
            }
        )
    return in_maps


def kernel(x, wq, wk, wv, w_proj, b_proj):
    x = np.asarray(x, dtype=np.float32)
    wq = np.asarray(wq, dtype=np.float32)
    wk = np.asarray(wk, dtype=np.float32)
    wv = np.asarray(wv, dtype=np.float32)
    w_proj = np.asarray(w_proj, dtype=np.float32)
    b_proj = np.asarray(b_proj, dtype=np.float32)

    nc = _build()
    in_maps = _make_in_maps(x, wq, wk, wv, w_proj)
    res = run_bass_kernel_spmd(nc, in_maps, core_ids=list(range(NCORES)))
    acc = np.zeros((B, T, C), dtype=np.float64)
    for c, r in enumerate(res.results):
        acc[c // 4] += r["out"]
    return (acc + b_proj).astype(np.float32)


# revision 36
# speedup vs baseline: 1.0040x; 1.0040x over previous
"""Multi-head causal attention (B=2, T=2048, C=1024, H=16, HS=64) on 8 TRN2
NeuronCores.

Sharding: hybrid batch x head-group tensor parallel. Core c handles batch
c//4 and heads 4*(c%4) .. 4*(c%4)+3, processed as two head-PAIRS (A, B)
that are software-pipelined against each other. Each core loads only its
batch's activations once (bf16, SBUF-resident), computes a partial output
out_t[C, T] via its 256 rows of w_proj, and the host sums 4 partials per
batch (+ bias) and transposes.

Per-core kernel (bf16 matmuls, fp32 PSUM accumulate):
  - All inputs host-pre-arranged partition-major so each DMA descriptor
    moves a 4-32 KB contiguous row (the DMA was descriptor-bound, not
    bandwidth-bound, in the naive layout).
  - Warm-up matmuls on the first-arriving weight sliver keep the PE HAM
    clock at 2.4 GHz through the initial DMA fill (and through known
    stall windows); a dummy exp preloads the ScalarE activation table.
  - QT/KT/VT [128(2 heads x 64), T] per pair: lhsT=weight chunks (bf16,
    FWL-fast weight load), rhs=resident xT chunks; 8-matmul runs hit the
    216 ns/matmul N=512 streaming floor.
  - V_aug [keys, 128]: V (cols 0:64 via PE-transpose of VT) | ones.
  - Flash-style causal attention in transposed layout: S^T[keys, q] via
    lhsT=KT block (K=64; the two heads' matmuls occupy disjoint PE row
    groups and overlap), exp on ScalarE, O^T accumulated via lhsT=V_aug
    — rows 64:128 give the softmax sums. OT for block-pair jg is emitted
    during jg+1 so the PE never head-of-line blocks on exp.
  - Generator-based schedule: each attention group's exp-bound steps are
    filled with QKV units of the other pair or (transposed-output)
    projection units, with explicit prerequisite drains.
  - PSUM is only read via copies (ALU ops reading PSUM mis-execute) and
    partition shifts only happen in copies (lane-wise ALU ops cannot
    shift partitions).
"""

import math
import sys
from contextlib import ExitStack

if "/opt/trn_rl_repo" not in sys.path:
    sys.path.insert(0, "/opt/trn_rl_repo")

import numpy as np

import concourse.mybir as mybir
import concourse.tile as tile
from concourse import bacc
from concourse.bass import ts
from concourse.bass_utils import run_bass_kernel_spmd

B, T, C = 2, 2048, 1024
H, HS = 16, 64
NCORES = 8
P = 128
G = 512  # q-group size
NG = T // G
KB = 128  # key block
NPO = C // P  # contraction chunks
F32 = mybir.dt.float32
BF16 = mybir.dt.bfloat16
EXP = mybir.ActivationFunctionType.Exp
MULT = mybir.AluOpType.mult

_nc_cache = {}


def _emit(tc):
    nc = tc.nc
    # All inputs are host-pre-arranged so every DMA descriptor moves a
    # 4-32 KB contiguous row (descriptor-count, not bandwidth, limits the
    # small strided layout).
    xt = nc.dram_tensor("xt", [P, NG, NPO, G], BF16, kind="ExternalInput").ap()
    wq2 = nc.dram_tensor("wq2", [P, NPO, 2 * P], BF16, kind="ExternalInput").ap()
    wk2 = nc.dram_tensor("wk2", [P, NPO, 2 * P], BF16, kind="ExternalInput").ap()
    wv2 = nc.dram_tensor("wv2", [P, NPO, 2 * P], BF16, kind="ExternalInput").ap()
    wptd = nc.dram_tensor("wpt", [P, 2, C], BF16, kind="ExternalInput").ap()
    trid = nc.dram_tensor("tri", [P, P], BF16, kind="ExternalInput").ap()
    identd = nc.dram_tensor("ident", [P, 64], BF16, kind="ExternalInput").ap()
    onesd = nc.dram_tensor("ones", [P, T // KB, 64], BF16, kind="ExternalInput").ap()
    out = nc.dram_tensor("out", [T, C], F32, kind="ExternalOutput").ap()

    ctx = ExitStack()
    persist = ctx.enter_context(tc.tile_pool(name="persist", bufs=1))
    qk_pool = ctx.enter_context(tc.tile_pool(name="qkp", bufs=2))
    vt_pool = ctx.enter_context(tc.tile_pool(name="vtp", bufs=2))
    vaug_pool = ctx.enter_context(tc.tile_pool(name="vaugp", bufs=2))
    pt_pool = ctx.enter_context(tc.tile_pool(name="ptp", bufs=3))
    norm_pool = ctx.enter_context(tc.tile_pool(name="normp", bufs=2))
    ohat_pool = ctx.enter_context(tc.tile_pool(name="ohatp", bufs=2))
    out_pool = ctx.enter_context(tc.tile_pool(name="outp", bufs=4))
    st_psum = ctx.enter_context(tc.tile_pool(name="stps", bufs=2, space="PSUM"))
    ot_psum = ctx.enter_context(tc.tile_pool(name="otps", bufs=2, space="PSUM"))
    mm_psum = ctx.enter_context(tc.tile_pool(name="mmps", bufs=2, space="PSUM"))

    xt_sb = persist.tile([P, NG, NPO, G], BF16, tag="xt")
    wq_sb = persist.tile([P, NPO, 2 * P], BF16, tag="wq")
    wk_sb = persist.tile([P, NPO, 2 * P], BF16, tag="wk")
    wv_sb = persist.tile([P, NPO, 2 * P], BF16, tag="wv")
    wpt_sb = persist.tile([P, 2, C], BF16, tag="wpt")
    tri_sb = persist.tile([P, P], BF16, tag="tri")
    ident = persist.tile([P, 64], BF16, tag="ident")

    # ---- DMA schedule: a sliver of wq first (enables warm-up), then xt ----
    nc.sync.dma_start(wq_sb[:, 0:2, :], wq2[:, 0:2, :])

    # PE warm-up during the DMA fill: keeps the HAM clock releasing to
    # 2.4 GHz before the first real matmul. Also preload the exp table.
    warm_ps = mm_psum.tile([P, G], F32, tag="mm", name="warm")
    for _ in range(15):
        nc.tensor.matmul(
            warm_ps[:], wq_sb[:, 0, 0:P], wq_sb[:, 0:2, :], start=True, stop=True
        )
    dummy_act = norm_pool.tile([P, 8], F32, tag="dact", name="dummy_act")
    nc.scalar.activation(dummy_act[:], wq_sb[:, 0, 0:8], EXP, scale=0.01)

    # xt in po-halves: subtile deps let each QKV matmul start as soon as
    # the half covering its contraction chunk has landed.
    nc.sync.dma_start(xt_sb[:, 0, 0:4, :], xt[:, 0, 0:4, :])
    nc.gpsimd.dma_start(xt_sb[:, 0, 4:NPO, :], xt[:, 0, 4:NPO, :])
    nc.gpsimd.dma_start(ident[:], identd[:])
    nc.gpsimd.dma_start(tri_sb[:], trid[:])
    nc.sync.dma_start(wq_sb[:, 2:NPO, :], wq2[:, 2:NPO, :])
    nc.sync.dma_start(wk_sb[:], wk2[:])
    nc.sync.dma_start(wv_sb[:], wv2[:])
    for tg in range(1, NG):
        nc.sync.dma_start(xt_sb[:, tg, 0:4, :], xt[:, tg, 0:4, :])
        nc.gpsimd.dma_start(xt_sb[:, tg, 4:NPO, :], xt[:, tg, 4:NPO, :])

    def new_pair(p):
        # ones DMAs (needed only by this pair's first OT) are deferred out
        # of the startup window via emit_ones.
        st = {
            "p": p,
            "qt": qk_pool.tile([P, T], BF16, tag="qt", name=f"qt{p}"),
            "kt": qk_pool.tile([P, T], BF16, tag="kt", name=f"kt{p}"),
            "vt": vt_pool.tile([P, T], BF16, tag="vt", name=f"vt{p}"),
            "ohat": ohat_pool.tile([P, T], BF16, tag="ohat", name=f"oh{p}"),
            "vaug": [
                vaug_pool.tile([P, T // KB, 128], BF16, tag=f"vaug{h}", name=f"va{p}{h}")
                for h in range(2)
            ],
        }
        return st

    def emit_ones(st):
        for h in range(2):
            nc.gpsimd.dma_start(st["vaug"][h][:, :, 64:128], onesd[:])

    stA = new_pair(0)
    stB = new_pair(1)
    emit_ones(stA)

    def emit_qkv(st, which, tg):
        w_sb, dst = {
            "q": (wq_sb, st["qt"]),
            "k": (wk_sb, st["kt"]),
            "v": (wv_sb, st["vt"]),
        }[which]
        sl = slice(P * st["p"], P * st["p"] + P)
        ps = mm_psum.tile([P, G], F32, tag="mm", name=f"qkv{which}{tg}")
        for po in range(NPO):
            nc.tensor.matmul(
                ps[:],
                w_sb[:, po, sl],
                xt_sb[:, tg, po, :],
                start=(po == 0),
                stop=(po == NPO - 1),
            )
        nc.vector.tensor_copy(dst[:, ts(tg, G)], ps[:])

    def emit_vaug(st, tg):
        tps = [
            mm_psum.tile([P, 4, 64], BF16, tag="mm", name=f"vtr{h}")
            for h in range(2)
        ]
        for kk in range(4):
            kb = 4 * tg + kk
            for h in range(2):
                nc.tensor.transpose(
                    tps[h][:, kk, :],
                    st["vt"][64 * h : 64 * h + 64, ts(kb, KB)],
                    ident[64 * h : 64 * h + 64, :],
                )
        for h in range(2):
            nc.vector.tensor_copy(st["vaug"][h][:, 4 * tg : 4 * tg + 4, 0:64], tps[h][:])

    def emit_ot(st, g, otps, js, pt_h):
        n_j = 4 * g + 4
        for idx, j in enumerate(js):
            r = j - 4 * g
            q0 = P * r if r >= 0 else 0
            for h in range(2):
                nc.tensor.matmul(
                    otps[h][:, q0:G],
                    st["vaug"][h][:, j, :],
                    pt_h[h][:, idx, q0:G],
                    start=(j == 0),
                    stop=(j == n_j - 1),
                )

    def gen_attn(st, g):
        p, qt, kt, ohat = st["p"], st["qt"], st["kt"], st["ohat"]
        n_j = 4 * g + 4
        njg = (n_j + 1) // 2
        otps = [
            ot_psum.tile([P, G], F32, tag="ot", name=f"ot{p}{g}{h}") for h in range(2)
        ]
        prev = None
        for jg in range(njg):
            js = [j for j in (2 * jg, 2 * jg + 1) if j < n_j]
            diag = 2 * jg >= 4 * g
            stps_h = [
                st_psum.tile([P, 2, G], F32, tag="st", name=f"st{p}{g}{h}")
                for h in range(2)
            ]
            pt_h = [
                pt_pool.tile([P, 2, G], BF16, tag=f"pt{h}", name=f"pt{p}{g}{h}")
                for h in range(2)
            ]
            for idx, j in enumerate(js):
                r = j - 4 * g
                q0 = P * r if r >= 0 else 0
                for h in range(2):
                    hb = 64 * h
                    nc.tensor.matmul(
                        stps_h[h][:, idx, q0:G],
                        kt[hb : hb + 64, ts(j, KB)],
                        qt[hb : hb + 64, G * g + q0 : G * (g + 1)],
                        start=True,
                        stop=True,
                    )
            for h in range(2):
                if not diag:
                    nc.scalar.activation(
                        pt_h[h][:, :, :],
                        stps_h[h][:, :, :],
                        EXP,
                        scale=float(HS) ** -0.5,
                    )
                else:
                    # one exp per (jg, head): the extra [q00:q01) slice of
                    # idx 1 is stale-but-bounded PSUM whose pt columns the
                    # OT matmul never reads.
                    q00 = P * (js[0] - 4 * g)
                    nc.scalar.activation(
                        pt_h[h][:, :, q00:G],
                        stps_h[h][:, :, q00:G],
                        EXP,
                        scale=float(HS) ** -0.5,
                    )
                    for idx, j in enumerate(js):
                        q0 = P * (j - 4 * g)
                        nc.vector.tensor_tensor(
                            pt_h[h][:, idx, q0 : q0 + P],
                            pt_h[h][:, idx, q0 : q0 + P],
                            tri_sb[:],
                            MULT,
                        )
            if prev is not None:
                emit_ot(st, g, otps, *prev)
            prev = (js, pt_h)
            yield
        emit_ot(st, g, otps, *prev)
        # normalization: PSUM is only ever read via copies (ALU ops reading
        # PSUM directly mis-execute); copies also handle partition shifts.
        l_sb = norm_pool.tile([P, G], F32, tag="lsb", name=f"l{p}{g}")
        rinv = norm_pool.tile([P, G], F32, tag="rinv", name=f"r{p}{g}")
        stag = norm_pool.tile([P, G], F32, tag="stag", name=f"sg{p}{g}")
        for h in range(2):
            hb = 64 * h
            nc.scalar.copy(l_sb[hb : hb + 64, :], otps[h][64:128, :])
            nc.vector.tensor_copy(stag[hb : hb + 64, :], otps[h][0:64, :])
        nc.vector.reciprocal_approx_fast(rinv[:], l_sb[:])
        nc.vector.tensor_tensor(ohat[:, ts(g, G)], stag[:], rinv[:], MULT)
        yield

    def gen_qkv_tg(st, tg):
        for which in ("q", "k", "v"):
            emit_qkv(st, which, tg)
            yield
        emit_vaug(st, tg)
        yield

    def gen_proj(tg):
        # lhsT = ohat t-chunk (stationary, reused across both c-halves),
        # rhs = w_projT moving: halves the proj LDWEIGHTS count and gives
        # the output in [T, C] orientation directly.
        for tc4 in range(G // P):
            t0 = G * tg + P * tc4
            pss = [
                mm_psum.tile([P, G], F32, tag="mm", name=f"pj{tg}{tc4}{n}")
                for n in range(2)
            ]
            for pp in range(2):
                for n in range(2):
                    nc.tensor.matmul(
                        pss[n][:],
                        [stA, stB][pp]["ohat"][:, t0 : t0 + P],
                        wpt_sb[:, pp, ts(n, G)],
                        start=(pp == 0),
                        stop=(pp == 1),
                    )
            for n in range(2):
                o_sb = out_pool.tile([P, G], F32, tag="osb", name=f"osb{tg}{tc4}{n}")
                # mid-kernel: keep ScalarE free for exp; at the tail split
                # copies across engines to drain faster.
                if tg == 3 and n == 1:
                    nc.scalar.copy(o_sb[:], pss[n][:])
                else:
                    nc.vector.tensor_copy(o_sb[:], pss[n][:])
                if n == 0:
                    nc.sync.dma_start(out[t0 : t0 + P, ts(n, G)], o_sb[:])
                else:
                    nc.gpsimd.dma_start(out[t0 : t0 + P, ts(n, G)], o_sb[:])
            yield

    def advance(gens):
        while gens:
            try:
                next(gens[0])
                return
            except StopIteration:
                gens.pop(0)

    def drain(gens):
        while gens:
            advance(gens)

    # ---- pipelined schedule: attention groups start as soon as their
    # inputs exist; later QKV and proj units serve as PE filler inside
    # each group's exp-bound steps. Every group's prerequisite QKV units
    # are force-drained before the group starts. ----
    fill = []

    def attn_with_fill(st, g, late=None):
        # `late` generators join the fill only after the first step, so a
        # unit stalling on the previous group's norm can't head-of-line
        # block this group's first score matmuls.
        for i, _ in enumerate(gen_attn(st, g)):
            advance(fill)
            if i == 0 and late:
                fill.extend(late)

    qA = [gen_qkv_tg(stA, tg) for tg in range(NG)]
    qB = [gen_qkv_tg(stB, tg) for tg in range(NG)]
    pj = [gen_proj(tg) for tg in range(NG)]

    def emit_warm(n):
        # Filler matmuls on already-loaded weights: executed in queue order,
        # they occupy the PE during xt-DMA stalls and keep the HAM clock hot.
        wps = mm_psum.tile([P, G], F32, tag="mm", name="warmf")
        for _ in range(n):
            nc.tensor.matmul(
                wps[:], wq_sb[:, 0, 0:P], wq_sb[:, 0:2, :], start=True, stop=True
            )

    drain([qA[0]])
    emit_warm(8)
    fill.append(qA[1])
    attn_with_fill(stA, 0)
    drain([qA[1]])
    emit_warm(6)
    emit_ones(stB)
    fill.append(qA[2])
    attn_with_fill(stA, 1)
    drain([qA[2]])
    fill.append(qA[3])
    attn_with_fill(stA, 2)
    drain([qA[3]])
    nc.gpsimd.dma_start(wpt_sb[:], wptd[:])
    fill.append(qB[0])
    fill.append(qB[1])
    attn_with_fill(stA, 3)
    drain([qB[0]])
    fill.append(qB[2])
    fill.append(qB[3])
    attn_with_fill(stB, 0)
    drain([qB[1]])
    attn_with_fill(stB, 1, late=[pj[0]])
    drain([qB[2]])
    attn_with_fill(stB, 2, late=[pj[1]])
    drain([qB[3]])
    attn_with_fill(stB, 3, late=[pj[2]])
    fill.append(pj[3])
    drain(fill)

    ctx.close()


def _build():
    if "nc" in _nc_cache:
        return _nc_cache["nc"]
    nc = bacc.Bacc("TRN2", target_bir_lowering=False, debug=False)
    with tile.TileContext(nc) as tc:
        _emit(tc)
    nc.compile()
    _nc_cache["nc"] = nc
    return nc


def _make_in_maps(x, wq, wk, wv, w_proj):
    import ml_dtypes

    bf = ml_dtypes.bfloat16
    tri = np.triu(np.ones((P, P), dtype=np.float32)).astype(bf)
    ident = np.tile(np.eye(64, dtype=np.float32), (2, 1)).astype(bf)
    ones = np.ones((P, T // KB, 64), dtype=np.float32).astype(bf)

    def part_major(a, blocks):
        # [blocks*P, cols] -> [P, blocks, cols] so each partition's DMA
        # row is one contiguous span.
        cols = a.shape[1]
        return np.ascontiguousarray(
            a.reshape(blocks, P, cols).transpose(1, 0, 2)
        ).astype(bf)

    xts = []
    for b in range(B):
        xT = np.asarray(x[b], np.float32).T  # [C, T]
        xts.append(
            np.ascontiguousarray(
                xT.reshape(NPO, P, NG, G).transpose(1, 2, 0, 3)
            ).astype(bf)
        )
    in_maps = []
    for c in range(NCORES):
        b, hp = divmod(c, 4)
        h0 = 4 * hp
        in_maps.append(
            {
                "xt": xts[b],
                "wq2": part_major(
                    np.concatenate([wq[h0 + i] for i in range(4)], axis=1), NPO
                ),
                "wk2": part_major(
                    np.concatenate([wk[h0 + i] for i in range(4)], axis=1), NPO
                ),
                "wv2": part_major(
                    np.concatenate([wv[h0 + i] for i in range(4)], axis=1), NPO
                ),
                "wpt": part_major(
                    np.ascontiguousarray(w_proj[:, 256 * hp : 256 * (hp + 1)].T), 2
                ),
                "tri": tri,
                "ident": ident,
                "ones": ones,
            }
        )
    return in_maps


def kernel(x, wq, wk, wv, w_proj, b_proj):
    x = np.asarray(x, dtype=np.float32)
    wq = np.asarray(wq, dtype=np.float32)
    wk = np.asarray(wk, dtype=np.float32)
    wv = np.asarray(wv, dtype=np.float32)
    w_proj = np.asarray(w_proj, dtype=np.float32)
    b_proj = np.asarray(b_proj, dtype=np.float32)

    nc = _build()
    in_maps = _make_in_maps(x, wq, wk, wv, w_proj)
    res = run_bass_kernel_spmd(nc, in_maps, core_ids=list(range(NCORES)))
    acc = np.zeros((B, T, C), dtype=np.float64)
    for c, r in enumerate(res.results):
        acc[c // 4] += r["out"]
    return (acc + b_proj).astype(np.float32)
